# revision 3
# baseline (speedup 1.0000x reference)
"""Trainium2 Bass kernel for MessagePassingWithPhase (8 axon trn2 cores).

Measured: the ~50us/instruction platform cost is per STATIC program record
(+ ~100us per loop iteration); loop bodies re-execute near HW speed. So the
message-matmul fills (8 mm + 1 drain) and the rank-8 gate accumulation
(product + add) run inside For_i loops with dynamic slices, cutting static
records per iteration from ~220 to ~70.

Measured platform law (axon terminal): execution is effectively serial at
~40-60us PER INSTRUCTION (any engine); tile-pool allocations add ~30us; the
matmul moving operand is ISA-capped at 512 columns. So v7 minimizes TOTAL
instruction count (~220/iter vs ~360 for v1):

  * All work tiles are persistent (allocated once, rewritten in place).
  * One PSUM tile [D, 4096]; matmuls fill it in 8x512 chunks.
  * Hidden build: ONE broadcast-add TT + ONE in-place relu per batch
    ([D, 64*512] bf16).
  * Gate: rank-8 on DVE (16 big TT/stt ops per batch, no PE), using
    host-prebroadcast phase rows (cb) and host-computed per-receiver
    weights (wt8); mask+bg fused into the last op; ONE in-place sigmoid
    per batch ([D, 32768]).
  * b2 folded out of the per-edge path: agg = sum(msg*G) + b2*sum(G),
    with sum(G) one segmented reduce per batch.
  * Messages: 64 matmuls per batch (irreducible), consumed straight from
    PSUM by multiply+segmented-reduce pairs (no drain step).
"""
import os
import sys
import numpy as np

for _p in ("/opt/trn_rl_repo", "/root/.axon_site/_ro/trn_rl_repo"):
    if os.path.isdir(_p) and _p not in sys.path:
        sys.path.append(_p)

B, N, D, O = 2, 512, 128, 4
NCORES = 8
NPC = N // NCORES       # 64 receivers per core
MASK_NEG = -48.0        # exact in fp8e4m3
FCH = 8                 # matmuls per PSUM fill (4096 cols)

MM_DT = "bfloat16"      # message matmul dtype (H and W2)
REPEAT = 1

_CACHE = {}


def _build_program():
    import concourse.bacc as bacc
    import concourse.mybir as mybir
    import concourse.tile as tile
    from concourse.bass import ts as dslice

    f32 = mybir.dt.float32
    bf16 = mybir.dt.bfloat16
    mmd = getattr(mybir.dt, MM_DT)
    fp8 = mybir.dt.float8e4
    A = mybir.AluOpType
    AF = mybir.ActivationFunctionType

    nc = bacc.Bacc("TRN2", debug=False)

    def din(name, shape, dt=f32):
        return nc.declare_dram_parameter(name, list(shape), dt, isOutput=False)

    xt = din("xt", (B, D, N))
    xtr = din("xtr", (D, B * NPC))
    cb = din("cb", (D, B * 2 * O * N), bf16)   # prebroadcast phase rows
    wt8 = din("wt8", (D, B * NPC * 2 * O))     # wg2[o,d]*p8r[b][o,i]
    mneg = din("mneg", (1, NPC * N), fp8)      # -48*(1-mask), (i,j) flat
    NBLOB = 6 * D + B * NPC + 5
    blob = din("blob", (D, NBLOB))
    out = nc.declare_dram_parameter("out", [B, D, NPC], f32, isOutput=True)

    with tile.TileContext(nc) as tc:
        with (
            tc.tile_pool(name="const", bufs=1) as cp,
            tc.tile_pool(name="psA", bufs=1, space="PSUM") as psA,
        ):
            def ct(dram, shape, dt=f32, tag=None):
                t = cp.tile(list(shape), dt, tag=tag, name=tag)
                nc.sync.dma_start(t[:], dram[:])
                return t

            blob_t = ct(blob, (D, NBLOB), tag="blob")
            w1r_t = blob_t[:, 0 * D: 1 * D]
            w1s_t = blob_t[:, 1 * D: 2 * D]
            w2_f = blob_t[:, 2 * D: 3 * D]
            wu1x_t = blob_t[:, 3 * D: 4 * D]
            wu1a_t = blob_t[:, 4 * D: 5 * D]
            wu2_t = blob_t[:, 5 * D: 6 * D]
            cinv_t = blob_t[:, 6 * D: 6 * D + B * NPC]
            bofs = 6 * D + B * NPC
            b1c_t = blob_t[:, bofs + 0: bofs + 1]
            bgc_t = blob_t[:, bofs + 1: bofs + 2]
            b2c_t = blob_t[:, bofs + 2: bofs + 3]
            bu1c_t = blob_t[:, bofs + 3: bofs + 4]
            bu2c_t = blob_t[:, bofs + 4: bofs + 5]

            w2b = cp.tile([D, D], mmd, tag="w2b", name="w2b")
            nc.vector.tensor_copy(w2b[:], w2_f)

            xt_t = [ct(xt[b], (D, N), tag=f"xt{b}") for b in range(B)]
            xtr_all = ct(xtr, (D, B * NPC), tag="xtr")
            cb_t = ct(cb, (D, B * 2 * O * N), bf16, tag="cb")
            wt8_t = ct(wt8, (D, B * NPC * 2 * O), tag="wt8")
            mnb = cp.tile([D, NPC * N], fp8, tag="mnb", name="mnb")
            nc.sync.dma_start(
                mnb[:], mneg[0:1, :].broadcast_to((D, NPC * N)))

            # persistent work tiles
            Hbuf = cp.tile([D, NPC * N], mmd, tag="Hbuf", name="Hbuf")
            glin = cp.tile([D, NPC * N], bf16, tag="glin", name="glin")
            araw = cp.tile([D, B * NPC], f32, tag="araw", name="araw")
            sumG = cp.tile([D, B * NPC], f32, tag="sumG", name="sumG")
            sendb = cp.tile([D, B * N], mmd, tag="sendb", name="sendb")
            recvb = cp.tile([D, B * NPC], f32, tag="recvb", name="recvb")
            smA = cp.tile([D, B * NPC], f32, tag="smA", name="smA")
            smB = cp.tile([D, B * NPC], f32, tag="smB", name="smB")
            ps = psA.tile([D, FCH * N], f32, tag="pp", name="pp")

            def cbs(b, o):       # [D, N] broadcast-resident phase row
                off = (b * 2 * O + o) * N
                return cb_t[:, off: off + N]


            for rep in range(REPEAT):
                # projections (PSUM sub-regions of the single tile)
                for b in range(B):
                    nc.tensor.matmul(ps[:, 0:N], w1s_t, xt_t[b][:],
                                     start=True, stop=True)
                    nc.vector.tensor_copy(sendb[:, b * N: (b + 1) * N],
                                          ps[:, 0:N])
                nc.tensor.matmul(ps[:, N:N + B * NPC], w1r_t, xtr_all[:],
                                 start=True, stop=True)
                nc.vector.tensor_scalar(recvb[:], ps[:, N:N + B * NPC],
                                        b1c_t, None, op0=A.add)

                g3 = glin[:].rearrange("p (a c) -> p a c", a=NPC)
                h3 = Hbuf[:].rearrange("p (a c) -> p a c", a=NPC)
                with tc.For_i(0, B) as bv:
                    # ----- gate: glin = sum_o cb_o * wt_o  (rank-8, For_i) -----
                    nc.vector.memset(glin[:], 0.0)
                    with tc.For_i(0, 2 * O) as o:
                        nc.vector.tensor_tensor(
                            h3,
                            cb_t[:, dslice(o + bv * 2 * O, N)].unsqueeze(1)
                                .broadcast_to((D, NPC, N)),
                            wt8_t[:, dslice(o + bv * 2 * O, NPC)].unsqueeze(2)
                                .broadcast_to((D, NPC, N)),
                            A.mult)
                        nc.vector.tensor_tensor(glin[:], glin[:], Hbuf[:],
                                                A.add)
                    # + mask(-48 off-neighbors) + bg, one fused op
                    nc.vector.scalar_tensor_tensor(
                        glin[:], mnb[:], bgc_t, glin[:],
                        op0=A.add, op1=A.add)
                    # ONE in-place sigmoid per batch
                    nc.scalar.activation(glin[:], glin[:], AF.Sigmoid)
                    # sum of gates per receiver (for the b2 term)
                    nc.vector.reduce_sum(sumG[:, dslice(bv, NPC)], g3,
                                         axis=mybir.AxisListType.X)
                    # ----- hidden: H = relu(send_j + recv_i) -----
                    nc.vector.tensor_tensor(
                        h3,
                        sendb[:, dslice(bv, N)].unsqueeze(1)
                            .broadcast_to((D, NPC, N)),
                        recvb[:, dslice(bv, NPC)].unsqueeze(2)
                            .broadcast_to((D, NPC, N)),
                        A.add)
                    nc.vector.tensor_scalar(Hbuf[:], Hbuf[:], 0.0, None,
                                            op0=A.max)
                    # --- messages: For_i over PSUM fills, drain into Hbuf ---
                    with tc.For_i(0, NPC // FCH) as f:
                        for k in range(FCH):
                            nc.tensor.matmul(
                                ps[:, k * N: (k + 1) * N], w2b[:],
                                Hbuf[:, dslice(f * FCH + k, N)],
                                start=True, stop=True)
                        nc.vector.tensor_copy(
                            Hbuf[:, dslice(f, FCH * N)], ps[:])
                    # gate-multiply and masked sum, one whole-batch op each
                    nc.vector.tensor_tensor(Hbuf[:], Hbuf[:], glin[:], A.mult)
                    nc.vector.reduce_sum(
                        araw[:, dslice(bv, NPC)], h3,
                        axis=mybir.AxisListType.X)

                # ----- tail: agg, update net, residual -----
                nc.vector.scalar_tensor_tensor(
                    smA[:], sumG[:], b2c_t, araw[:], op0=A.mult, op1=A.add)
                nc.vector.tensor_tensor(smA[:], smA[:], cinv_t, A.mult)
                nc.tensor.matmul(ps[:, 0: B * NPC], wu1x_t, xtr_all[:],
                                 start=True, stop=False)
                nc.tensor.matmul(ps[:, 0: B * NPC], wu1a_t, smA[:],
                                 start=False, stop=True)
                nc.vector.tensor_scalar(smB[:], ps[:, 0: B * NPC], bu1c_t,
                                        0.0, op0=A.add, op1=A.max)
                nc.tensor.matmul(ps[:, B * NPC: 2 * B * NPC], wu2_t, smB[:],
                                 start=True, stop=True)
                nc.vector.scalar_tensor_tensor(
                    smA[:], ps[:, B * NPC: 2 * B * NPC], bu2c_t, xtr_all[:],
                    op0=A.add, op1=A.add)
                nc.sync.dma_start(out[:].rearrange("b d n -> d b n"),
                                  smA[:].rearrange("d (b n) -> d b n", b=B))

    nc.compile()
    return nc


def _get_program():
    key = (MM_DT, REPEAT, FCH)
    if key not in _CACHE:
        _CACHE[key] = _build_program()
    return _CACHE[key]


def kernel(node_features, node_phases, adjacency,
           W1r, W1s, b1, W2, b2, Wg, bg, Wu1x, Wu1a, bu1, Wu2, bu2,
           _trace=False, _trace_kwargs=None):
    from concourse import bass_utils
    import ml_dtypes

    f4 = np.float32
    bf = ml_dtypes.bfloat16
    f8 = ml_dtypes.float8_e4m3
    x = np.asarray(node_features, f4)
    ph = np.asarray(node_phases, f4)
    adj = np.asarray(adjacency)

    mask = (adj != 0)
    counts = np.maximum(mask.sum(axis=1), 1).astype(f4)
    cinv_full = (1.0 / counts)

    mneg_full = (MASK_NEG * (~mask)).astype(f4)     # (N, N)

    xt_full = np.ascontiguousarray(x.transpose(0, 2, 1))
    p8_full = np.ascontiguousarray(
        np.concatenate([np.cos(ph), np.sin(ph)], axis=2).transpose(0, 2, 1))
    wg2 = np.concatenate([np.asarray(Wg, f4), np.asarray(Wg, f4)], axis=0)

    cb_full = np.broadcast_to(
        p8_full.reshape(1, B * 2 * O * N), (D, B * 2 * O * N)).astype(bf)

    common = dict(xt=xt_full, cb=np.ascontiguousarray(cb_full))

    in_maps = []
    for c in range(NCORES):
        lo, hi = c * NPC, (c + 1) * NPC
        m = dict(common)
        m["xtr"] = np.ascontiguousarray(
            np.concatenate([xt_full[b][:, lo:hi] for b in range(B)], axis=1))
        p8r = p8_full[:, :, lo:hi]                   # (B, 8, NPC)
        wt = np.einsum('od,boi->dboi', wg2, p8r)     # (D, B, 8, NPC)
        m["wt8"] = np.ascontiguousarray(wt.reshape(D, B * NPC * 2 * O), f4)
        m["mneg"] = np.ascontiguousarray(
            mneg_full[lo:hi, :].reshape(1, NPC * N)).astype(f8)
        cinvb = np.broadcast_to(cinv_full[lo:hi][None, :], (D, NPC))
        m["blob"] = np.ascontiguousarray(np.concatenate(
            [np.asarray(W1r, f4), np.asarray(W1s, f4), np.asarray(W2, f4),
             np.asarray(Wu1x, f4), np.asarray(Wu1a, f4), np.asarray(Wu2, f4),
             cinvb, cinvb,
             np.asarray(b1, f4).reshape(D, 1), np.asarray(bg, f4).reshape(D, 1),
             np.asarray(b2, f4).reshape(D, 1), np.asarray(bu1, f4).reshape(D, 1),
             np.asarray(bu2, f4).reshape(D, 1)], axis=1))
        in_maps.append(m)

    nc = _get_program()
    res = bass_utils.run_bass_kernel_spmd(
        nc, in_maps, list(range(NCORES)),
        trace=_trace, **(_trace_kwargs or {}))

    out = np.empty((B, N, D), f4)
    for c in range(NCORES):
        lo, hi = c * NPC, (c + 1) * NPC
        out[:, lo:hi, :] = res.results[c]["out"].transpose(0, 2, 1)

    kernel.last_results = res
    return out
